# revision 4
# baseline (speedup 1.0000x reference)
"""Local causal (sliding-window) attention on 8 Trainium2 NeuronCores.

Strategy: sequence-parallel. Each core owns 512 consecutive query tokens of
one batch element (cores 0-3 -> batch 0, cores 4-7 -> batch 1) plus a
128-token halo of preceding tokens whose k/v are recomputed locally, so no
inter-core communication is needed. The dataflow is fully transposed
(features on partitions, tokens on the free dim) so no on-device transposes
are required: the host pre-transposes each core's x-shard and transposes the
per-core output back while gathering. All matmuls run in float32r (fp22) at
full rate with moving dims >= 256.
"""

import sys

sys.path.insert(0, "/opt/trn_rl_repo")
import numpy as np

B, S, D = 2, 2048, 1024
H, DH = 16, 64
WINDOW = 128
NCORES = 8
SLOC = 512            # queries per core
HALO = 128
TLOC = SLOC + HALO    # 640 local tokens (halo + queries)
NST = 2               # query supertiles of 256 per core
CPB = NCORES // B     # cores per batch element

_cached = {}


def _build():
    import concourse.bacc as bacc
    import concourse.mybir as mybir
    import concourse.tile as tile

    f32 = mybir.dt.float32
    f32r = mybir.dt.float32r
    AF = mybir.ActivationFunctionType

    nc = bacc.Bacc(None)
    xT_d = nc.declare_dram_parameter("xT", [D, TLOC], f32, isOutput=False)
    wqkv_d = nc.declare_dram_parameter("w_qkv", [D, 3 * D], f32, isOutput=False)
    wout_d = nc.declare_dram_parameter("w_out", [D, D], f32, isOutput=False)
    mask_d = nc.declare_dram_parameter("mask", [NST, 3, 128, 256], f32, isOutput=False)
    ones_d = nc.declare_dram_parameter("ones", [128, H], f32, isOutput=False)
    out_d = nc.declare_dram_parameter("outT", [D, SLOC], f32, isOutput=True)

    with tile.TileContext(nc) as tc:
        with (
            tc.tile_pool(name="sb", bufs=1) as sb,
            tc.tile_pool(name="wqp", bufs=1) as wqp,
            tc.tile_pool(name="work", bufs=1) as work,
        ):
            # ---- persistent SBUF tiles ----
            xt = [sb.tile([128, TLOC], f32r, tag=f"xt{k}", name=f"xt{k}") for k in range(8)]
            for k in range(8):
                nc.sync.dma_start(out=xt[k][:], in_=xT_d[k * 128:(k + 1) * 128, :].bitcast(f32r))
            msk = [
                sb.tile([128, 256], f32, tag=f"mk{i}", name=f"mk{i}") for i in range(NST * 3)
            ]
            for st in range(NST):
                for r in range(3):
                    nc.sync.dma_start(out=msk[st * 3 + r][:], in_=mask_d[st, r])
            wo = [sb.tile([128, D], f32r, tag=f"wo{k}", name=f"wo{k}") for k in range(8)]
            for k in range(8):
                nc.sync.dma_start(out=wo[k][:], in_=wout_d[k * 128:(k + 1) * 128, :].bitcast(f32r))
            ones_sb = sb.tile([128, H], f32r, tag="ones", name="ones_sb")
            nc.sync.dma_start(out=ones_sb[:], in_=ones_d[:].bitcast(f32r))

            qT = [sb.tile([128, SLOC], f32r, tag=f"qT{i}", name=f"qT{i}") for i in range(8)]
            kT = [sb.tile([128, TLOC], f32r, tag=f"kT{i}", name=f"kT{i}") for i in range(8)]
            # v in natural [token, head-major] layout with a ones column per head
            vt = [sb.tile([128, 65 * H], f32r, tag=f"v{t}", name=f"v{t}") for t in range(5)]
            for t in range(5):
                v_ones = vt[t].rearrange("p (h c) -> p h c", c=65)[:, :, 64]
                nc.vector.tensor_copy(v_ones, ones_sb[:])
            att = [sb.tile([128, SLOC], f32r, tag=f"at{t}", name=f"at{t}") for t in range(8)]

            # ---- phase 1: qkv projection, streamed over 6 column blocks ----
            with tc.tile_pool(name="qkps", bufs=2, space="PSUM") as qkps:
                for cb in range(6):
                    wq = []
                    for k in range(8):
                        wqk = wqp.tile([128, 512], f32r, tag="wq", bufs=16, name=f"wq{cb}_{k}")
                        nc.sync.dma_start(
                            out=wqk[:],
                            in_=wqkv_d[k * 128:(k + 1) * 128, cb * 512:(cb + 1) * 512].bitcast(f32r),
                        )
                        wq.append(wqk)
                    if cb < 2:
                        # q columns -> qT rows cb*512..+512 (queries only: tokens HALO..TLOC)
                        for m in range(4):
                            ps = qkps.tile([128, 512], f32, tag="qk", name=f"psq{cb}_{m}")
                            for k in range(8):
                                nc.tensor.matmul(
                                    ps[:],
                                    wq[k][:, m * 128:(m + 1) * 128],
                                    xt[k][:, HALO:TLOC],
                                    start=(k == 0),
                                    stop=(k == 7),
                                )
                            nc.scalar.copy(qT[cb * 4 + m][:], ps[:])
                    elif cb < 4:
                        # k columns -> kT rows (cb-2)*512..+512, all 640 tokens
                        for m in range(4):
                            for n in range(2):
                                ps = qkps.tile([128, 320], f32, tag="qk", name=f"psk{cb}_{m}_{n}")
                                for k in range(8):
                                    nc.tensor.matmul(
                                        ps[:],
                                        wq[k][:, m * 128:(m + 1) * 128],
                                        xt[k][:, n * 320:(n + 1) * 320],
                                        start=(k == 0),
                                        stop=(k == 7),
                                    )
                                nc.scalar.copy(
                                    kT[(cb - 2) * 4 + m][:, n * 320:(n + 1) * 320], ps[:]
                                )
                    else:
                        # v columns, natural layout: tokens on partitions
                        for t in range(5):
                            ps = qkps.tile([128, 512], f32, tag="qk", name=f"psv{cb}_{t}")
                            for k in range(8):
                                nc.tensor.matmul(
                                    ps[:],
                                    xt[k][:, t * 128:(t + 1) * 128],
                                    wq[k][:, :],
                                    start=(k == 0),
                                    stop=(k == 7),
                                )
                            h0 = (cb - 4) * 8
                            dst = vt[t].rearrange("p (h c) -> p h c", c=65)[:, h0:h0 + 8, 0:64]
                            src = ps[:].rearrange("p (h c) -> p h c", c=64)
                            nc.scalar.copy(dst, src)

            # ---- phase 2+3: attention and output projection ----
            with (
                tc.tile_pool(name="scps", bufs=3, space="PSUM") as scps,
                tc.tile_pool(name="avps", bufs=2, space="PSUM") as avps,
                tc.tile_pool(name="opps", bufs=2, space="PSUM") as opps,
            ):
                for st in range(NST):
                    for h in range(H):
                        t, poff = h // 2, (h % 2) * 64
                        ptiles = []
                        for r in range(3):
                            jb = st * 2 + r
                            ssc = scps.tile([128, 256], f32, tag="sc", name=f"sc{st}_{h}_{r}")
                            nc.tensor.matmul(
                                ssc[:],
                                kT[t][poff:poff + 64, jb * 128:(jb + 1) * 128],
                                qT[t][poff:poff + 64, st * 256:(st + 1) * 256],
                                start=True,
                                stop=True,
                            )
                            nc.vector.tensor_add(ssc[:], ssc[:], msk[st * 3 + r][:])
                            pt = work.tile([128, 256], f32r, tag="p", bufs=6, name=f"p{st}_{h}_{r}")
                            nc.scalar.activation(pt[:], ssc[:], AF.Exp, scale=0.125)
                            ptiles.append(pt)
                        av = avps.tile([65, 256], f32, tag="av", name=f"av{st}_{h}")
                        for r in range(3):
                            nc.tensor.matmul(
                                av[:],
                                vt[st * 2 + r][:, h * 65:h * 65 + 65],
                                ptiles[r][:],
                                start=(r == 0),
                                stop=(r == 2),
                            )
                        nc.vector.tensor_copy(
                            att[t][poff:poff + 64, st * 256:(st + 1) * 256], av[0:64, :]
                        )
                        rrow = work.tile([1, 256], f32, tag="rr", bufs=4, name=f"rr{st}_{h}")
                        nc.vector.reciprocal(rrow[:], av[64:65, :])
                        rb = work.tile([128, 256], f32, tag="rb", bufs=4, name=f"rb{st}_{h}")
                        nc.gpsimd.partition_broadcast(rb[:], rrow[:])
                        asl = att[t][poff:poff + 64, st * 256:(st + 1) * 256]
                        nc.vector.tensor_mul(asl, asl, rb[poff:poff + 64, :])
                    # output projection for this supertile
                    for m in range(8):
                        po = opps.tile([128, 256], f32, tag="op", name=f"po{st}_{m}")
                        for k in range(8):
                            nc.tensor.matmul(
                                po[:],
                                wo[k][:, m * 128:(m + 1) * 128],
                                att[k][:, st * 256:(st + 1) * 256],
                                start=(k == 0),
                                stop=(k == 7),
                            )
                        ot = work.tile([128, 256], f32, tag="ot", bufs=4, name=f"ot{st}_{m}")
                        nc.scalar.copy(ot[:], po[:])
                        nc.sync.dma_start(
                            out=out_d[m * 128:(m + 1) * 128, st * 256:(st + 1) * 256],
                            in_=ot[:],
                        )

    nc.finalize()
    return nc


def _get_nc():
    if "nc" not in _cached:
        _cached["nc"] = _build()
    return _cached["nc"]


def _core_inputs(x, w_qkv, w_out):
    in_maps = []
    for c in range(NCORES):
        b, qs = c // CPB, (c % CPB) * SLOC
        xs = np.zeros((TLOC, D), dtype=np.float32)
        lo = max(0, qs - HALO)
        xs[HALO - (qs - lo):] = x[b, lo:qs + SLOC]
        # masks: [st, r, j, i] additive bias on raw scores (exp applies 0.125 scale)
        i = np.arange(256)[None, None, None, :]
        j = np.arange(128)[None, None, :, None]
        st = np.arange(NST)[:, None, None, None]
        r = np.arange(3)[None, :, None, None]
        qg = qs + st * 256 + i
        kg = qs + st * 256 - HALO + r * 128 + j
        allowed = (kg <= qg) & (kg > qg - WINDOW) & (kg >= 0)
        mask = np.where(allowed, 0.0, -8e30).astype(np.float32)
        in_maps.append(
            {
                "xT": np.ascontiguousarray(xs.T),
                "w_qkv": np.ascontiguousarray(w_qkv, dtype=np.float32),
                "w_out": np.ascontiguousarray(w_out, dtype=np.float32),
                "mask": mask,
                "ones": np.ones((128, H), dtype=np.float32),
            }
        )
    return in_maps


def kernel(x, w_qkv, w_out, _trace=False, _trace_kwargs=None):
    from concourse.bass_utils import run_bass_kernel_spmd

    x = np.asarray(x, dtype=np.float32)
    w_qkv = np.asarray(w_qkv, dtype=np.float32)
    w_out = np.asarray(w_out, dtype=np.float32)
    nc = _get_nc()
    in_maps = _core_inputs(x, w_qkv, w_out)
    res = run_bass_kernel_spmd(
        nc, in_maps, list(range(NCORES)), trace=_trace, **(_trace_kwargs or {})
    )
    out = np.concatenate(
        [res.results[c]["outT"].T for c in range(NCORES)], axis=0
    ).reshape(B, S, D)
    if _trace:
        return out, res
    return out


# revision 11
# speedup vs baseline: 1.0856x; 1.0856x over previous
"""Local causal (sliding-window) attention on 8 Trainium2 NeuronCores.

Strategy: sequence-parallel. Each core owns 512 consecutive query tokens of
one batch element (cores 0-3 -> batch 0, cores 4-7 -> batch 1) plus a
128-token halo of preceding tokens whose k/v are recomputed locally, so no
inter-core communication is needed. The dataflow is fully transposed
(features on partitions, tokens on the free dim) so no on-device transposes
are required: the host pre-transposes each core's x-shard and transposes the
per-core output back while gathering. All matmuls run in float32r (fp22) at
full rate with moving dims >= 256.
"""

import sys

sys.path.insert(0, "/opt/trn_rl_repo")
import numpy as np

B, S, D = 2, 2048, 1024
H, DH = 16, 64
WINDOW = 128
NCORES = 8
SLOC = 512            # queries per core
HALO = 128
TLOC = SLOC + HALO    # 640 local tokens (halo + queries)
NST = 2               # query supertiles of 256 per core
CPB = NCORES // B     # cores per batch element

_cached = {}


def _build():
    import concourse.bacc as bacc
    import concourse.mybir as mybir
    import concourse.tile as tile

    f32 = mybir.dt.float32
    f32r = mybir.dt.float32r
    AF = mybir.ActivationFunctionType

    nc = bacc.Bacc(None)
    xT_d = nc.declare_dram_parameter("xT", [D, TLOC], f32, isOutput=False)
    wqkv_d = nc.declare_dram_parameter("w_qkv", [D, 3 * D], f32, isOutput=False)
    wout_d = nc.declare_dram_parameter("w_out", [D, D], f32, isOutput=False)
    mask_d = nc.declare_dram_parameter("mask", [NST, 2, 128, 256], f32, isOutput=False)
    ones_d = nc.declare_dram_parameter("ones", [128, H], f32, isOutput=False)
    out_d = nc.declare_dram_parameter("outT", [D, SLOC], f32, isOutput=True)

    with tile.TileContext(nc) as tc:
        with tc.tile_pool(name="sb", bufs=1) as sb:
            # ---- persistent SBUF tiles ----
            ph1 = tc.alloc_tile_pool(name="ph1", bufs=1)
            xt = [ph1.tile([128, TLOC], f32r, tag=f"xt{k}", name=f"xt{k}") for k in range(8)]
            for k in range(8):
                nc.sync.dma_start(out=xt[k][:], in_=xT_d[k * 128:(k + 1) * 128, :].bitcast(f32r))
            msk = [
                sb.tile([128, 256], f32, tag=f"mk{i}", name=f"mk{i}") for i in range(NST * 2)
            ]
            for st in range(NST):
                for r in range(2):
                    nc.sync.dma_start(out=msk[st * 2 + r][:], in_=mask_d[st, r])
            wo = [sb.tile([128, D], f32r, tag=f"wo{k}", name=f"wo{k}") for k in range(8)]
            for k in range(8):
                nc.sync.dma_start(out=wo[k][:], in_=wout_d[k * 128:(k + 1) * 128, :].bitcast(f32r))
            ones_sb = sb.tile([128, H], f32r, tag="ones", name="ones_sb")
            nc.sync.dma_start(out=ones_sb[:], in_=ones_d[:].bitcast(f32r))

            qT = [sb.tile([128, SLOC], f32r, tag=f"qT{i}", name=f"qT{i}") for i in range(8)]
            kT = [sb.tile([128, TLOC], f32r, tag=f"kT{i}", name=f"kT{i}") for i in range(8)]
            # v in natural [token, head-major] layout with a ones column per head
            vt = [sb.tile([128, 65 * H], f32r, tag=f"v{t}", name=f"v{t}") for t in range(5)]
            for t in range(5):
                v_ones = vt[t].rearrange("p (h c) -> p h c", c=65)[:, :, 64]
                nc.vector.tensor_copy(v_ones, ones_sb[:])
            att = [sb.tile([128, SLOC], f32r, tag=f"at{t}", name=f"at{t}") for t in range(8)]

            # ---- phase 1: qkv projection, streamed over 6 column blocks ----
            with tc.tile_pool(name="qkps", bufs=2, space="PSUM") as qkps:
                for cb in range(6):
                    wq = []
                    for k in range(8):
                        wqk = ph1.tile([128, 512], f32r, tag="wq", bufs=16, name=f"wq{cb}_{k}")
                        nc.sync.dma_start(
                            out=wqk[:],
                            in_=wqkv_d[k * 128:(k + 1) * 128, cb * 512:(cb + 1) * 512].bitcast(f32r),
                        )
                        wq.append(wqk)
                    if cb < 2:
                        # q columns -> qT rows cb*512..+512 (queries only: tokens HALO..TLOC)
                        for m in range(4):
                            ps = qkps.tile([128, 512], f32, tag="qk", name=f"psq{cb}_{m}")
                            for k in range(8):
                                nc.tensor.matmul(
                                    ps[:],
                                    wq[k][:, m * 128:(m + 1) * 128],
                                    xt[k][:, HALO:TLOC],
                                    start=(k == 0),
                                    stop=(k == 7),
                                )
                            nc.scalar.copy(qT[cb * 4 + m][:], ps[:])
                    elif cb < 4:
                        # k columns -> kT rows (cb-2)*512..+512, all 640 tokens
                        for m in range(4):
                            for n in range(2):
                                ps = qkps.tile([128, 320], f32, tag="qk", name=f"psk{cb}_{m}_{n}")
                                for k in range(8):
                                    nc.tensor.matmul(
                                        ps[:],
                                        wq[k][:, m * 128:(m + 1) * 128],
                                        xt[k][:, n * 320:(n + 1) * 320],
                                        start=(k == 0),
                                        stop=(k == 7),
                                    )
                                nc.scalar.copy(
                                    kT[(cb - 2) * 4 + m][:, n * 320:(n + 1) * 320], ps[:]
                                )
                    else:
                        # v columns, natural layout: tokens on partitions
                        for t in range(5):
                            ps = qkps.tile([128, 512], f32, tag="qk", name=f"psv{cb}_{t}")
                            for k in range(8):
                                nc.tensor.matmul(
                                    ps[:],
                                    xt[k][:, t * 128:(t + 1) * 128],
                                    wq[k][:, :],
                                    start=(k == 0),
                                    stop=(k == 7),
                                )
                            h0 = (cb - 4) * 8
                            dst = vt[t].rearrange("p (h c) -> p h c", c=65)[:, h0:h0 + 8, 0:64]
                            src = ps[:].rearrange("p (h c) -> p h c", c=64)
                            nc.scalar.copy(dst, src)

            ph1.release()
            work = tc.alloc_tile_pool(name="work", bufs=1)

            # ---- phase 2+3: attention and output projection ----
            # per (st, h): scores for the 3 key blocks packed as two [128,256]
            # psums: sc02 = [r0 x queries 0:128 | r2 x queries 128:256], sc1 = r1.
            # AV accumulates the same three pieces into one [65,256] psum whose
            # row 64 (from the ones column of v) is the softmax denominator.
            DEPTH = 4
            with (
                tc.tile_pool(name="scps", bufs=1, space="PSUM") as scps,
                tc.tile_pool(name="aops", bufs=1, space="PSUM") as aops,
            ):
                scat = work.tile([1, H * 256], f32, tag="scat", name="scat")
                rcat = work.tile([1, H * 256], f32, tag="rcat", name="rcat")
                for st in range(NST):
                    q0 = st * 256
                    pend = {}
                    avs = {}

                    def emit_qk(h):
                        t, poff = h // 2, (h % 2) * 64
                        jb = st * 2
                        sc0 = scps.tile([128, 128], f32, tag="sch", bufs=4, name=f"sc0_{st}_{h}")
                        nc.tensor.matmul(
                            sc0[:],
                            kT[t][poff:poff + 64, jb * 128:(jb + 1) * 128],
                            qT[t][poff:poff + 64, q0:q0 + 128],
                            start=True, stop=True,
                        )
                        sc2 = scps.tile([128, 128], f32, tag="sch", bufs=4, name=f"sc2_{st}_{h}")
                        nc.tensor.matmul(
                            sc2[:],
                            kT[t][poff:poff + 64, (jb + 2) * 128:(jb + 3) * 128],
                            qT[t][poff:poff + 64, q0 + 128:q0 + 256],
                            start=True, stop=True,
                        )
                        sc1 = scps.tile([128, 256], f32, tag="sc", bufs=2, name=f"sc1_{st}_{h}")
                        nc.tensor.matmul(
                            sc1[:],
                            kT[t][poff:poff + 64, (jb + 1) * 128:(jb + 2) * 128],
                            qT[t][poff:poff + 64, q0:q0 + 256],
                            start=True, stop=True,
                        )
                        nc.vector.tensor_add(sc0[:], sc0[:], msk[st * 2 + 0][:, 0:128])
                        nc.vector.tensor_add(sc2[:], sc2[:], msk[st * 2 + 0][:, 128:256])
                        nc.vector.tensor_add(sc1[:], sc1[:], msk[st * 2 + 1][:])
                        p0 = work.tile([128, 128], f32r, tag="p0", bufs=DEPTH + 2, name=f"p0_{st}_{h}")
                        p2 = work.tile([128, 128], f32r, tag="p2", bufs=DEPTH + 2, name=f"p2_{st}_{h}")
                        p1 = work.tile([128, 256], f32r, tag="p1", bufs=DEPTH + 2, name=f"p1_{st}_{h}")
                        nc.scalar.activation(p0[:], sc0[:], AF.Exp, scale=0.125)
                        nc.scalar.activation(p2[:], sc2[:], AF.Exp, scale=0.125)
                        nc.scalar.activation(p1[:], sc1[:], AF.Exp, scale=0.125)
                        pend[h] = (p0, p1, p2)

                    def emit_av(h):
                        t, poff = h // 2, (h % 2) * 64
                        jb = st * 2
                        p0, p1, p2 = pend.pop(h)
                        avl = aops.tile([65, 128], f32, tag="ao", bufs=2, name=f"avl{st}_{h}")
                        nc.tensor.matmul(
                            avl[:], vt[jb][:, h * 65:h * 65 + 65], p0[:],
                            start=True, stop=False,
                        )
                        nc.tensor.matmul(
                            avl[:], vt[jb + 1][:, h * 65:h * 65 + 65], p1[:, 0:128],
                            start=False, stop=True,
                        )
                        avr = aops.tile([65, 128], f32, tag="ao", bufs=2, name=f"avr{st}_{h}")
                        nc.tensor.matmul(
                            avr[:], vt[jb + 1][:, h * 65:h * 65 + 65], p1[:, 128:256],
                            start=True, stop=False,
                        )
                        nc.tensor.matmul(
                            avr[:], vt[jb + 2][:, h * 65:h * 65 + 65], p2[:],
                            start=False, stop=True,
                        )
                        nc.scalar.copy(scat[0:1, h * 256:h * 256 + 128], avl[64:65, :])
                        nc.scalar.copy(scat[0:1, h * 256 + 128:h * 256 + 256], avr[64:65, :])
                        nc.vector.tensor_copy(att[t][poff:poff + 64, q0:q0 + 128], avl[0:64, :])
                        nc.vector.tensor_copy(att[t][poff:poff + 64, q0 + 128:q0 + 256], avr[0:64, :])
                        avs[h] = None

                    for step in range(H + DEPTH):
                        if step < H:
                            emit_qk(step)
                        if step >= DEPTH:
                            emit_av(step - DEPTH)

                    # batched softmax denominators: spread sums to 16 partitions,
                    # one reciprocal, spread back, broadcast per head.
                    s16 = work.tile([16, 256], f32, tag="s16", bufs=2, name=f"s16_{st}")
                    for h in range(H):
                        nc.sync.dma_start(out=s16[h:h + 1, :], in_=scat[0:1, h * 256:(h + 1) * 256])
                    r16 = work.tile([16, 256], f32, tag="r16", bufs=2, name=f"r16_{st}")
                    nc.vector.reciprocal(r16[:], s16[:])
                    for h in range(H):
                        nc.sync.dma_start(out=rcat[0:1, h * 256:(h + 1) * 256], in_=r16[h:h + 1, :])
                    for h in range(H):
                        t, poff = h // 2, (h % 2) * 64
                        rb = work.tile([128, 256], f32, tag="rb", bufs=4, name=f"rb{st}_{h}")
                        nc.gpsimd.partition_broadcast(rb[:], rcat[0:1, h * 256:(h + 1) * 256])
                        asl = att[t][poff:poff + 64, q0:q0 + 256]
                        nc.vector.tensor_mul(asl, asl, rb[poff:poff + 64, :])
                    # output projection for this supertile
                    for m in range(8):
                        po = aops.tile([128, 256], f32, tag="ao", bufs=2, name=f"po{st}_{m}")
                        for k in range(8):
                            nc.tensor.matmul(
                                po[:],
                                wo[k][:, m * 128:(m + 1) * 128],
                                att[k][:, q0:q0 + 256],
                                start=(k == 0),
                                stop=(k == 7),
                            )
                        ot = work.tile([128, 256], f32, tag="ot", bufs=4, name=f"ot{st}_{m}")
                        nc.scalar.copy(ot[:], po[:])
                        nc.sync.dma_start(
                            out=out_d[m * 128:(m + 1) * 128, q0:q0 + 256],
                            in_=ot[:],
                        )
            work.release()

    nc.finalize()
    return nc


def _get_nc():
    if "nc" not in _cached:
        _cached["nc"] = _build()
    return _cached["nc"]


def _core_inputs(x, w_qkv, w_out):
    in_maps = []
    for c in range(NCORES):
        b, qs = c // CPB, (c % CPB) * SLOC
        xs = np.zeros((TLOC, D), dtype=np.float32)
        lo = max(0, qs - HALO)
        xs[HALO - (qs - lo):] = x[b, lo:qs + SLOC]
        # masks: additive bias on raw scores (exp applies the 0.125 scale).
        # mask[st][0] packs [r0 x queries 0:128 | r2 x queries 128:256];
        # mask[st][1] is r1 (middle key block) for all 256 queries.
        i = np.arange(256)[None, None, None, :]
        j = np.arange(128)[None, None, :, None]
        st = np.arange(NST)[:, None, None, None]
        r = np.arange(3)[None, :, None, None]
        qg = qs + st * 256 + i
        kg = qs + st * 256 - HALO + r * 128 + j
        allowed = (kg <= qg) & (kg > qg - WINDOW) & (kg >= 0)
        m3 = np.where(allowed, 0.0, -8e30).astype(np.float32)  # [NST, 3, 128, 256]
        mask = np.empty((NST, 2, 128, 256), dtype=np.float32)
        mask[:, 0, :, 0:128] = m3[:, 0, :, 0:128]
        mask[:, 0, :, 128:256] = m3[:, 2, :, 128:256]
        mask[:, 1] = m3[:, 1]
        in_maps.append(
            {
                "xT": np.ascontiguousarray(xs.T),
                "w_qkv": np.ascontiguousarray(w_qkv, dtype=np.float32),
                "w_out": np.ascontiguousarray(w_out, dtype=np.float32),
                "mask": mask,
                "ones": np.ones((128, H), dtype=np.float32),
            }
        )
    return in_maps


def kernel(x, w_qkv, w_out, _trace=False, _trace_kwargs=None):
    from concourse.bass_utils import run_bass_kernel_spmd

    x = np.asarray(x, dtype=np.float32)
    w_qkv = np.asarray(w_qkv, dtype=np.float32)
    w_out = np.asarray(w_out, dtype=np.float32)
    nc = _get_nc()
    in_maps = _core_inputs(x, w_qkv, w_out)
    res = run_bass_kernel_spmd(
        nc, in_maps, list(range(NCORES)), trace=_trace, **(_trace_kwargs or {})
    )
    out = np.concatenate(
        [res.results[c]["outT"].T for c in range(NCORES)], axis=0
    ).reshape(B, S, D)
    if _trace:
        return out, res
    return out


# revision 13
# speedup vs baseline: 1.2261x; 1.1294x over previous
"""Local causal (sliding-window) attention on 8 Trainium2 NeuronCores.

Strategy: sequence-parallel. Each core owns 512 consecutive query tokens of
one batch element (cores 0-3 -> batch 0, cores 4-7 -> batch 1) plus a
128-token halo of preceding tokens whose k/v are recomputed locally, so no
inter-core communication is needed. The dataflow is fully transposed
(features on partitions, tokens on the free dim) so no on-device transposes
are required: the host pre-transposes each core's x-shard and transposes the
per-core output back while gathering. All matmuls run in float32r (fp22) at
full rate.

Attention per (supertile st of 256 queries, head h): the 256-query window
spans 3 key blocks r0/r1/r2 of 128 tokens. Masks are DVE-copied into PSUM
first and the QK matmuls accumulate onto them (start=False); the fully
masked quadrants of r0/r2 are never computed (half-width matmuls). exp runs
on ScalarE into SBUF p-tiles; AV accumulates v^T p with an extra ones
column in v producing the softmax denominators, which take a DMA round trip
through a 16-partition tile for one batched reciprocal, then a GpSimd
partition-broadcast feeds the normalizing multiply.
"""

import sys

sys.path.insert(0, "/opt/trn_rl_repo")
import numpy as np

B, S, D = 2, 2048, 1024
H, DH = 16, 64
WINDOW = 128
NCORES = 8
SLOC = 512            # queries per core
HALO = 128
TLOC = SLOC + HALO    # 640 local tokens (halo + queries)
NST = 2               # query supertiles of 256 per core
CPB = NCORES // B     # cores per batch element

_cached = {}


def _build():
    import concourse.bacc as bacc
    import concourse.mybir as mybir
    import concourse.tile as tile

    f32 = mybir.dt.float32
    f32r = mybir.dt.float32r
    bf16 = mybir.dt.bfloat16
    AF = mybir.ActivationFunctionType

    nc = bacc.Bacc(None)
    xT_d = nc.declare_dram_parameter("xT", [D, TLOC], f32, isOutput=False)
    wqkv_d = nc.declare_dram_parameter("w_qkv", [D, 3 * D], f32, isOutput=False)
    wout_d = nc.declare_dram_parameter("w_out", [D, D], f32, isOutput=False)
    mask_d = nc.declare_dram_parameter("mask", [NST, 2, 128, 256], f32, isOutput=False)
    eye_d = nc.declare_dram_parameter("eye", [128, 128], f32, isOutput=False)
    ones_d = nc.declare_dram_parameter("ones", [128, H], f32, isOutput=False)
    out_d = nc.declare_dram_parameter("outT", [D, SLOC], f32, isOutput=True)

    with tile.TileContext(nc) as tc:
        with (
            tc.tile_pool(name="sb", bufs=1) as sb,
            tc.tile_pool(name="qkps", bufs=1, space="PSUM") as qkps,
            tc.tile_pool(name="scps", bufs=1, space="PSUM") as scps,
            tc.tile_pool(name="aops", bufs=1, space="PSUM") as aops,
        ):
            # ---- persistent SBUF tiles; DMA order: xt+ones, wq stream, masks, wout
            xt = [sb.tile([128, TLOC], f32r, tag=f"xt{k}", name=f"xt{k}") for k in range(8)]
            for k in range(8):
                nc.sync.dma_start(out=xt[k][:], in_=xT_d[k * 128:(k + 1) * 128, :].bitcast(f32r))
            ones_sb = sb.tile([128, H], f32r, tag="ones", name="ones_sb")
            nc.sync.dma_start(out=ones_sb[:], in_=ones_d[:].bitcast(f32r))

            qT = [sb.tile([128, SLOC], f32r, tag=f"qT{i}", name=f"qT{i}") for i in range(8)]
            kT = [sb.tile([128, TLOC], f32r, tag=f"kT{i}", name=f"kT{i}") for i in range(8)]
            vt = [sb.tile([128, 65 * H], f32r, tag=f"v{t}", name=f"v{t}") for t in range(5)]
            for t in range(5):
                v_ones = vt[t].rearrange("p (h c) -> p h c", c=65)[:, :, 64]
                nc.vector.tensor_copy(v_ones, ones_sb[:])
            att = [sb.tile([128, SLOC], f32r, tag=f"at{t}", name=f"at{t}") for t in range(8)]

            def wq_dma(cb):
                tiles = []
                for k in range(8):
                    wqk = sb.tile([128, 512], f32r, tag="wq", bufs=16, name=f"wq{cb}_{k}")
                    nc.sync.dma_start(
                        out=wqk[:],
                        in_=wqkv_d[k * 128:(k + 1) * 128, cb * 512:(cb + 1) * 512].bitcast(f32r),
                    )
                    tiles.append(wqk)
                return tiles

            # ---- phase 1: qkv projection ----
            for cb in range(2):            # q columns; queries only
                wq = wq_dma(cb)
                for m in range(4):
                    ps = qkps.tile([128, 512], f32, tag="qk", bufs=2, name=f"psq{cb}_{m}")
                    for k in range(8):
                        nc.tensor.matmul(
                            ps[:], wq[k][:, m * 128:(m + 1) * 128], xt[k][:, HALO:TLOC],
                            start=(k == 0), stop=(k == 7),
                        )
                    nc.scalar.copy(qT[cb * 4 + m][:], ps[:])
            for cb in range(2, 4):         # k columns; all 640 tokens
                wq = wq_dma(cb)
                for m in range(4):
                    for n in range(2):
                        ps = qkps.tile([128, 320], f32, tag="qk", bufs=2, name=f"psk{cb}_{m}_{n}")
                        for k in range(8):
                            nc.tensor.matmul(
                                ps[:], wq[k][:, m * 128:(m + 1) * 128],
                                xt[k][:, n * 320:(n + 1) * 320],
                                start=(k == 0), stop=(k == 7),
                            )
                        nc.scalar.copy(kT[(cb - 2) * 4 + m][:, n * 320:(n + 1) * 320], ps[:])
            # v columns: token-tile-major across both column halves so vt[t]
            # completes in jb order for the attention pipeline
            wq4 = wq_dma(4)
            wq5 = wq_dma(5)
            msk = [sb.tile([128, 256], f32r, tag=f"mk{i}", name=f"mk{i}") for i in range(NST * 2)]
            eye_sb = sb.tile([128, 128], f32r, tag="eye", name="eye_sb")
            nc.sync.dma_start(out=eye_sb[:], in_=eye_d[:].bitcast(f32r))
            for st in range(NST):
                for r in range(2):
                    nc.sync.dma_start(out=msk[st * 2 + r][:], in_=mask_d[st, r].bitcast(f32r))
            for t in range(5):
                for half, wq in ((0, wq4), (1, wq5)):
                    ps = qkps.tile([128, 512], f32, tag="qk", bufs=2, name=f"psv{t}_{half}")
                    for k in range(8):
                        nc.tensor.matmul(
                            ps[:], xt[k][:, t * 128:(t + 1) * 128], wq[k][:, :],
                            start=(k == 0), stop=(k == 7),
                        )
                    h0 = half * 8
                    dst = vt[t].rearrange("p (h c) -> p h c", c=65)[:, h0:h0 + 8, 0:64]
                    src = ps[:].rearrange("p (h c) -> p h c", c=64)
                    nc.scalar.copy(dst, src)
            # w_out reuses the streamed-weight slots: half A = cols 0:512 of row
            # block k (proj m 0..3), half B = cols 512:1024 (m 4..7)
            woA = []
            woB = []
            for k in range(8):
                wa = sb.tile([128, 512], f32r, tag="wq", bufs=16, name=f"woA{k}")
                nc.sync.dma_start(out=wa[:], in_=wout_d[k * 128:(k + 1) * 128, 0:512].bitcast(f32r))
                woA.append(wa)
            for k in range(8):
                wb = sb.tile([128, 512], f32r, tag="wq", bufs=16, name=f"woB{k}")
                nc.sync.dma_start(out=wb[:], in_=wout_d[k * 128:(k + 1) * 128, 512:1024].bitcast(f32r))
                woB.append(wb)

            # ---- phase 2+3: attention and output projection ----
            DEPTH = 3
            scat = sb.tile([1, H * 256], f32, tag="scat", name="scat")
            rcat = sb.tile([1, H * 256], f32, tag="rcat", name="rcat")
            for st in range(NST):
                q0 = st * 256
                pend = {}

                def emit_qk(h, st=st, q0=q0, pend=pend):
                    t, poff = h // 2, (h % 2) * 64
                    jb = st * 2
                    sc02 = scps.tile([128, 256], f32, tag="sc", bufs=4, name=f"sc02_{st}_{h}")
                    nc.tensor.matmul(
                        sc02[:], eye_sb[:], msk[st * 2 + 0][:],
                        start=True, stop=False, skip_group_check=True,
                    )
                    nc.tensor.matmul(
                        sc02[:, 0:128],
                        kT[t][poff:poff + 64, jb * 128:(jb + 1) * 128],
                        qT[t][poff:poff + 64, q0:q0 + 128],
                        start=False, stop=False, skip_group_check=True,
                    )
                    nc.tensor.matmul(
                        sc02[:, 128:256],
                        kT[t][poff:poff + 64, (jb + 2) * 128:(jb + 3) * 128],
                        qT[t][poff:poff + 64, q0 + 128:q0 + 256],
                        start=False, stop=True, skip_group_check=True,
                    )
                    sc1 = scps.tile([128, 256], f32, tag="sc", bufs=4, name=f"sc1_{st}_{h}")
                    nc.tensor.matmul(
                        sc1[:], eye_sb[:], msk[st * 2 + 1][:],
                        start=True, stop=False, skip_group_check=True,
                    )
                    nc.tensor.matmul(
                        sc1[:],
                        kT[t][poff:poff + 64, (jb + 1) * 128:(jb + 2) * 128],
                        qT[t][poff:poff + 64, q0:q0 + 256],
                        start=False, stop=True, skip_group_check=True,
                    )
                    p02 = sb.tile([128, 256], f32r, tag="p02", bufs=DEPTH + 3, name=f"p02_{st}_{h}")
                    p1 = sb.tile([128, 256], f32r, tag="p1", bufs=DEPTH + 3, name=f"p1_{st}_{h}")
                    nc.scalar.activation(p02[:], sc02[:], AF.Exp, scale=0.125)
                    nc.scalar.activation(p1[:], sc1[:], AF.Exp, scale=0.125)
                    pend[h] = (p02, p1)

                def emit_av(h, st=st, q0=q0, pend=pend):
                    t, poff = h // 2, (h % 2) * 64
                    jb = st * 2
                    p02, p1 = pend.pop(h)
                    av = aops.tile([65, 256], f32, tag="ao", bufs=2, name=f"av{st}_{h}")
                    nc.tensor.matmul(
                        av[:], vt[jb + 1][:, h * 65:h * 65 + 65], p1[:],
                        start=True, stop=False, skip_group_check=True,
                    )
                    nc.tensor.matmul(
                        av[:, 0:128], vt[jb][:, h * 65:h * 65 + 65], p02[:, 0:128],
                        start=False, stop=False, skip_group_check=True,
                    )
                    nc.tensor.matmul(
                        av[:, 128:256], vt[jb + 2][:, h * 65:h * 65 + 65], p02[:, 128:256],
                        start=False, stop=True, skip_group_check=True,
                    )
                    nc.scalar.copy(scat[0:1, h * 256:(h + 1) * 256], av[64:65, :])
                    nc.vector.tensor_copy(att[t][poff:poff + 64, q0:q0 + 256], av[0:64, :])

                for step in range(H + DEPTH):
                    if step < H:
                        emit_qk(step)
                    if step >= DEPTH:
                        emit_av(step - DEPTH)

                # batched softmax denominators
                s16 = sb.tile([16, 256], f32, tag="s16", bufs=2, name=f"s16_{st}")
                for h in range(H):
                    nc.sync.dma_start(out=s16[h:h + 1, :], in_=scat[0:1, h * 256:(h + 1) * 256])
                r16 = sb.tile([16, 256], f32, tag="r16", bufs=2, name=f"r16_{st}")
                nc.vector.reciprocal(r16[:], s16[:])
                for h in range(H):
                    nc.sync.dma_start(out=rcat[0:1, h * 256:(h + 1) * 256], in_=r16[h:h + 1, :])
                for h in range(H):
                    t, poff = h // 2, (h % 2) * 64
                    rb = sb.tile([128, 256], f32, tag="rb", bufs=4, name=f"rb{st}_{h}")
                    nc.gpsimd.partition_broadcast(rb[:], rcat[0:1, h * 256:(h + 1) * 256])
                    asl = att[t][poff:poff + 64, q0:q0 + 256]
                    nc.vector.tensor_mul(asl, asl, rb[poff:poff + 64, :])
                # output projection for this supertile
                for m in range(8):
                    wo = woA if m < 4 else woB
                    mc = (m % 4) * 128
                    po = aops.tile([128, 256], f32, tag="ao", bufs=2, name=f"po{st}_{m}")
                    for k in range(8):
                        nc.tensor.matmul(
                            po[:], wo[k][:, mc:mc + 128], att[k][:, q0:q0 + 256],
                            start=(k == 0), stop=(k == 7),
                        )
                    ot = sb.tile([128, 256], f32, tag="ot", bufs=4, name=f"ot{st}_{m}")
                    nc.scalar.copy(ot[:], po[:])
                    nc.sync.dma_start(
                        out=out_d[m * 128:(m + 1) * 128, q0:q0 + 256], in_=ot[:],
                    )

    nc.finalize()
    return nc


def _get_nc():
    if "nc" not in _cached:
        _cached["nc"] = _build()
    return _cached["nc"]


def _core_inputs(x, w_qkv, w_out):
    in_maps = []
    for c in range(NCORES):
        b, qs = c // CPB, (c % CPB) * SLOC
        xs = np.zeros((TLOC, D), dtype=np.float32)
        lo = max(0, qs - HALO)
        xs[HALO - (qs - lo):] = x[b, lo:qs + SLOC]
        # masks: additive bias on raw scores (exp applies the 0.125 scale).
        # mask[st][0] packs [r0 x queries 0:128 | r2 x queries 128:256];
        # mask[st][1] is r1 (middle key block) for all 256 queries.
        i = np.arange(256)[None, None, None, :]
        j = np.arange(128)[None, None, :, None]
        st = np.arange(NST)[:, None, None, None]
        r = np.arange(3)[None, :, None, None]
        qg = qs + st * 256 + i
        kg = qs + st * 256 - HALO + r * 128 + j
        allowed = (kg <= qg) & (kg > qg - WINDOW) & (kg >= 0)
        m3 = np.where(allowed, 0.0, -8e30).astype(np.float32)
        mask = np.empty((NST, 2, 128, 256), dtype=np.float32)
        mask[:, 0, :, 0:128] = m3[:, 0, :, 0:128]
        mask[:, 0, :, 128:256] = m3[:, 2, :, 128:256]
        mask[:, 1] = m3[:, 1]
        in_maps.append(
            {
                "xT": np.ascontiguousarray(xs.T),
                "w_qkv": np.ascontiguousarray(w_qkv, dtype=np.float32),
                "w_out": np.ascontiguousarray(w_out, dtype=np.float32),
                "mask": mask,
                "ones": np.ones((128, H), dtype=np.float32),
                "eye": np.eye(128, dtype=np.float32),
            }
        )
    return in_maps


def kernel(x, w_qkv, w_out, _trace=False, _trace_kwargs=None):
    from concourse.bass_utils import run_bass_kernel_spmd

    x = np.asarray(x, dtype=np.float32)
    w_qkv = np.asarray(w_qkv, dtype=np.float32)
    w_out = np.asarray(w_out, dtype=np.float32)
    nc = _get_nc()
    in_maps = _core_inputs(x, w_qkv, w_out)
    res = run_bass_kernel_spmd(
        nc, in_maps, list(range(NCORES)), trace=_trace, **(_trace_kwargs or {})
    )
    out = np.concatenate(
        [res.results[c]["outT"].T for c in range(NCORES)], axis=0
    ).reshape(B, S, D)
    if _trace:
        return out, res
    return out
